# revision 14
# baseline (speedup 1.0000x reference)
"""GAT/GCN message-passing layer on 8 Trainium2 NeuronCores.

Math: per query node i the reference computes
    e[i,j] = f_src[i] + f_dst[j]   (masked by Ahat>0), attn = softmax_j, then
    out = relu(attn @ h_prime).
f_src[i] is constant along the softmax axis so it cancels; with g = exp(f_dst)
the layer collapses to one GEMM over the adjacency:
    out = relu( (Ahat @ [g*h' | g])[:, :256] / (Ahat @ [g*h' | g])[:, 256:] )
with h' = node_feats @ w and f_dst = node_feats @ (w @ w_a @ a[2:4]).

Sharding: 1D row partition of query nodes; each core owns 1024 output rows and
streams its [8192, 1024] adjacency slice (transposed so the contraction axis j
lands on SBUF partitions).  w/u/node_feats^T are replicated; every core
recomputes the B = [g*h' | g] panel locally.

Key speed tricks vs. the bf16 baseline:
  * The adjacency is binary, so fp8e4 is lossless for it: half the HBM bytes,
    and the whole 8.4MB/core slice fits pinned in SBUF (no streaming ring).
  * B is kept as an fp8 hi/lo pair (Blo = fp8(B - Bhi)), and the main GEMM
    runs in DoubleRow perf mode: lhsT = [A_2t | A_2t+1] (two j-blocks per
    instruction), rhs = [Bhi_2t | Bhi_2t+1] then [Blo_2t | Blo_2t+1].
    DoubleRow processes rows at 2x the bf16 rate, so the 257-col output
    streams in ~50ns instead of ~100ns; hi+lo together match bf16 accuracy
    (measured end-to-end rel-err ~2.2e-3).
  * All DRAM tensors are laid out as pre-tiled SBUF images (partition-major),
    so every DMA is 128 contiguous descriptors instead of 1024 - descriptor
    generation on the sequencers drops from ~8us to ~1us per load.
  * The prefix (h' panel) is interleaved with the main GEMM: prefix block j
    runs on PSUM banks 6/7 while main pairs t <= (j-3)/2 accumulate in banks
    0-5; i-blocks 6/7 are backfilled after the prefix finishes (all of A is
    resident, so the backfill is pure PE work).

walrus accepts only ONE sync wait per instruction, so the dataflow keeps each
instruction's cross-engine deps on a single engine: ACT (exp/Bhi) feeds DVE
(Blo, which reads Bhi so it transitively covers ACT), main-hi matmuls wait
only on ACT, main-lo only on DVE, and DMA first-touches are absorbed by PE
nops emitted at chunk boundaries.
"""

import sys

import ml_dtypes
import numpy as np

sys.path.insert(0, "/opt/trn_rl_repo")

import concourse.bass as bass  # noqa: E402
import concourse.tile as tile  # noqa: E402
from concourse import mybir  # noqa: E402
from concourse.bass_utils import run_bass_kernel_spmd  # noqa: E402
from concourse.tile import add_dep_helper  # noqa: E402

N = 8192
F = 256  # in_features == out_features
FE = F + 1  # h' columns + the g column
NCORES = 8
ROWS = N // NCORES  # 1024 output rows per core
P = 128
NJ = N // P  # 64 contraction blocks
NI = ROWS // P  # 8 output-row blocks per core
NPAIR = NJ // 2  # 32 DoubleRow j-block pairs

BF = mybir.dt.bfloat16
F8 = mybir.dt.float8e4
F32 = mybir.dt.float32

# params image column split: chunk0 = wext + nfT blocks 0..7, then 8..31, 32..63
PSPLIT = [0, FE + 8 * P, FE + 32 * P, FE + NJ * P]
# adjacency chunk split, in j-blocks: small first chunks so the first main
# pairs can start while params stream
ASPLIT = [0, 2, 8, 16, 24, 32, 40, 48, 56, 64]

_CACHE = {}


def _build():
    nc = bass.Bass(
        "TRN2",
        target_bir_lowering=False,
        debug=False,
        enable_asserts=True,
        num_devices=NCORES,
    )
    # pre-tiled images (partition-major; see _prep_inputs)
    aT = nc.dram_tensor("aT", [P, NJ, ROWS], F8, kind="ExternalInput").ap()
    p0 = nc.dram_tensor("p0", [P, 2, PSPLIT[1]], BF, kind="ExternalInput").ap()
    p1 = nc.dram_tensor(
        "p1", [P, 2, PSPLIT[2] - PSPLIT[1]], BF, kind="ExternalInput"
    ).ap()
    p2 = nc.dram_tensor(
        "p2", [P, 2, PSPLIT[3] - PSPLIT[2]], BF, kind="ExternalInput"
    ).ap()
    out = nc.dram_tensor("out", [P, NI, F], F32, kind="ExternalOutput").ap()

    with tile.TileContext(nc) as tc:
        _body(tc, aT, (p0, p1, p2), out)
    return nc


def _body(tc, aT, params, out):
    nc = tc.nc
    Exp = mybir.ActivationFunctionType.Exp
    Relu = mybir.ActivationFunctionType.Relu

    with (
        tc.tile_pool(name="consts", bufs=1) as consts,
        tc.tile_pool(name="rpool", bufs=8) as rpool,
        tc.tile_pool(name="psum", bufs=1, space="PSUM") as psum,
    ):
        # ---- SBUF tiles ----------------------------------------------------
        p_sb = [
            consts.tile([P, 2, PSPLIT[c + 1] - PSPLIT[c]], BF, tag=f"p{c}", name=f"p{c}")
            for c in range(3)
        ]
        aT_sb = consts.tile([P, NJ, ROWS], F8, tag="aT")
        Bhi = consts.tile([P, NJ, FE], F8, tag="Bhi")
        Blo = consts.tile([P, NJ, FE], F8, tag="Blo")
        G = consts.tile([P, NJ], F32, tag="G")  # g = exp(f_dst)
        # bf16 staging for B = g*h' (ACT writes it, DVE subtracts from it).
        # One slot per j-block: no reuse, so no WAR edge back to the DVE
        # reader (walrus allows only one sync wait per instruction).
        Bf = consts.tile([P, NJ, F], BF, tag="Bf")
        otile = consts.tile([P, NI * F], F32, tag="o")

        def nfT(j, kb):
            """SBUF [128, 128] lhsT view of node_feats^T block j, k-half kb."""
            if j < 8:
                return p_sb[0][:, kb, FE + j * P : FE + (j + 1) * P]
            if j < 32:
                return p_sb[1][:, kb, (j - 8) * P : (j - 7) * P]
            return p_sb[2][:, kb, (j - 32) * P : (j - 31) * P]

        wext = p_sb[0][:, :, 0:FE]

        # ---- loads ----------------------------------------------------------
        # all loads on SYNC (gpsimd's DMA queues stay virgin for the output
        # stores, which otherwise pick up a queue-reuse wait on top of their
        # data wait); each DMA is 128 contiguous descriptors.
        pdma = []
        prev = None
        for c in range(3):
            d = nc.sync.dma_start(p_sb[c][:], params[c][:])
            if prev is not None:
                add_dep_helper(d.ins, prev.ins, sync=False, reason="pdma order")
            prev = d
            pdma.append(d)
        adma = []
        for c in range(len(ASPLIT) - 1):
            lo, hi = ASPLIT[c], ASPLIT[c + 1]
            d = nc.sync.dma_start(aT_sb[:, lo:hi, :], aT[:, lo:hi, :])
            add_dep_helper(d.ins, prev.ins, sync=False, reason="adma order")
            prev = d
            adma.append(d)
        # keep params ahead of the adjacency bulk on the HBM fabric
        add_dep_helper(adma[2].ins, pdma[1].ins, reason="params first")
        add_dep_helper(adma[4].ins, pdma[2].ins, reason="params first")

        # ---- PSUM accumulators ----------------------------------------------
        acc = [psum.tile([P, FE], F32, tag=f"acc{i}", name=f"acc{i}") for i in range(NI)]
        # prefix h' borrows banks 6/7 (i-blocks 6/7 are backfilled later)
        hp = [acc[6], acc[7]]

        # first-touch absorbers: a PE nop waits on the DMA so the matmul that
        # follows needs only its ACT/DVE wait
        def pe_gate(dma):
            nop = nc.tensor.nop(nofuse=True, hint="dma_gate")
            add_dep_helper(nop.ins, dma.ins, reason="dma gate")
            return nop

        prev_act = None
        prev_dve = None
        last_mm = None

        def prefix(j):
            nonlocal prev_act, prev_dve, last_mm
            h = hp[j % 2]
            for kb in range(2):
                last_mm = nc.tensor.matmul(
                    h[:],
                    lhsT=nfT(j, kb),
                    rhs=wext[:, kb, :],
                    start=(kb == 0),
                    stop=(kb == 1),
                )
            # G[:, j] = exp(f_dst) = g  (the only ACT op that waits on PE here;
            # the Bf write's PE dep is dominated by it and elided)
            ex = nc.scalar.activation(G[:, j : j + 1], h[:, F : F + 1], Exp)
            if prev_act is not None:
                add_dep_helper(ex.ins, prev_act.ins, sync=False, reason="act order")
            # Bf = h' * g (f32 SBUF staging), Bhi = fp8(Bf) -- both ACT, so the
            # DVE Blo op below needs only the single ACT wait
            bfj = Bf[:, j, :]
            bf = nc.scalar.mul(bfj, h[:, 0:F], G[:, j : j + 1])
            add_dep_helper(bf.ins, ex.ins, sync=False, reason="act order")
            bh = nc.scalar.copy(Bhi[:, j, 0:F], bfj)
            add_dep_helper(bh.ins, bf.ins, sync=False, reason="act order")
            prev_act = bh
            # Blo[j] = fp8(Bf - Bhi[j])
            bl = nc.vector.tensor_tensor(
                Blo[:, j, 0:F], bfj, Bhi[:, j, 0:F], mybir.AluOpType.subtract
            )
            if prev_dve is not None:
                add_dep_helper(bl.ins, prev_dve.ins, sync=False, reason="dve order")
            prev_dve = bl
            if j % 4 == 3:
                # drop this 4-group's g columns into Bhi/Blo (strided fp8 casts)
                j0 = j - 3
                gh = nc.scalar.copy(Bhi[:, j0 : j + 1, F], G[:, j0 : j + 1])
                add_dep_helper(gh.ins, prev_act.ins, sync=False, reason="act order")
                prev_act = gh
                gl = nc.vector.tensor_tensor(
                    Blo[:, j0 : j + 1, F],
                    G[:, j0 : j + 1],
                    Bhi[:, j0 : j + 1, F],
                    mybir.AluOpType.subtract,
                )
                add_dep_helper(gl.ins, prev_dve.ins, sync=False, reason="dve order")
                prev_dve = gl

        def main_pair(t, ilist, start, stop):
            nonlocal last_mm
            for i in ilist:
                for half, Bt in ((0, Bhi), (1, Blo)):
                    last_mm = nc.tensor.matmul(
                        acc[i][:],
                        lhsT=aT_sb[:, 2 * t : 2 * t + 2, i * P : (i + 1) * P],
                        rhs=Bt[:, 2 * t : 2 * t + 2, :],
                        start=(start and half == 0),
                        stop=(stop and half == 1),
                        perf_mode=mybir.MatmulPerfMode.DoubleRow,
                    )

        # ---- interleaved prefix + main stream --------------------------------
        # adjacency chunk c covers pairs ASPLIT[c]/2 .. ASPLIT[c+1]/2 - 1
        a_gate = {ASPLIT[c] // 2: adma[c] for c in range(len(adma))}
        pe_gate(pdma[0])
        for j in range(NJ):
            if j == 8:
                pe_gate(pdma[1])
            if j == 32:
                pe_gate(pdma[2])
            prefix(j)
            if j >= 3 and j % 2 == 1:
                t = (j - 3) // 2  # pairs 0..30 during the prefix
                if t in a_gate:
                    pe_gate(a_gate[t])
                main_pair(t, range(6), start=(t == 0), stop=False)
        main_pair(31, range(6), start=False, stop=True)
        # backfill i-blocks 6/7 (banks 6/7 are free once the prefix drained)
        for t in range(NPAIR):
            main_pair(t, (6, 7), start=(t == 0), stop=(t == NPAIR - 1))

        # ---- epilogue: out[i] = relu(acc[i][:, :F] / acc[i][:, F]) ----------
        # banks 0..5 finish at the end of the interleaved stream and drain on
        # ACT while the backfill matmuls still run; banks 6/7 drain via DVE.
        stores = []
        banksA = list(range(6))
        denomA = rpool.tile([P, len(banksA)], F32, tag="denomA")
        denom_last = None
        for k, i in enumerate(banksA):
            dc = nc.scalar.copy(denomA[:, k : k + 1], acc[i][:, F : F + 1])
            if denom_last is not None:
                add_dep_helper(dc.ins, denom_last.ins, sync=False, reason="act order")
            denom_last = dc
        recipA = rpool.tile([P, len(banksA)], F32, tag="recipA")
        nc.vector.reciprocal(recipA[:], denomA[:])
        # sacrificial ACT read absorbs the DVE tick for the six fused relus
        sacA = rpool.tile([P, len(banksA)], F32, tag="sacA")
        sa = nc.scalar.copy(sacA[:], recipA[:])
        add_dep_helper(sa.ins, denom_last.ins, sync=False, reason="act order")
        last_relu = sa
        for k, i in enumerate(banksA):
            o = otile[:, i * F : (i + 1) * F]
            rl = nc.scalar.activation(o, acc[i][:, 0:F], Relu, scale=recipA[:, k : k + 1])
            add_dep_helper(rl.ins, last_relu.ins, sync=False, reason="act order")
            last_relu = rl
        # gpsimd nop absorbs the ACT dep so the store itself carries only its
        # DMA-queue wait
        gnop = nc.gpsimd.nop(nofuse=True, hint="storeA_gate")
        add_dep_helper(gnop.ins, last_relu.ins, reason="storeA gate")
        stores.append(nc.gpsimd.dma_start(out[:, 0:6, :], otile[:, 0 : 6 * F]))
        add_dep_helper(stores[-1].ins, gnop.ins, sync=False, reason="after gate")

        # banks 6/7 (backfill): DVE path
        denomB = rpool.tile([P, 2], F32, tag="denomB")
        for k, i in enumerate([6, 7]):
            dc = nc.scalar.copy(denomB[:, k : k + 1], acc[i][:, F : F + 1])
            add_dep_helper(dc.ins, (denom_last or dc).ins, sync=False, reason="act order")
            denom_last = dc
        recipB = rpool.tile([P, 2], F32, tag="recipB")
        nc.vector.reciprocal(recipB[:], denomB[:])
        rscrB = rpool.tile([P, 2], F32, tag="rscrB")
        nc.vector.tensor_copy(rscrB[:], recipB[:])
        last_dve = None
        for k, i in enumerate([6, 7]):
            o = otile[:, i * F : (i + 1) * F]
            nc.vector.tensor_scalar_mul(o, acc[i][:, 0:F], recipB[:, k : k + 1])
            last_dve = nc.vector.tensor_scalar_max(o, o, 0.0)
        gnop = nc.gpsimd.nop(nofuse=True, hint="storeB_gate")
        add_dep_helper(gnop.ins, last_dve.ins, reason="storeB gate")
        stores.append(nc.gpsimd.dma_start(out[:, 6:8, :], otile[:, 6 * F :]))
        add_dep_helper(stores[-1].ins, gnop.ins, sync=False, reason="after gate")

        # funnel every proc's final tick into SP via single-wait nops so the
        # kernel-tail drain has nothing left to wait on (every DMA queue's
        # final count included, else the drain aggregates 10+ waits)
        for dep in [
            *pdma,
            *adma,
            *stores,
            last_mm,
            last_relu,
            last_dve,
            prev_act,
            prev_dve,
        ]:
            nop = nc.sync.nop(nofuse=True, hint="tail_funnel")
            add_dep_helper(nop.ins, dep.ins, reason="tail funnel")


def _prep_inputs(node_feats, Ahat, w, w_a, a):
    node_feats = np.asarray(node_feats, dtype=np.float32)
    Ahat = np.asarray(Ahat, dtype=np.float32)
    w = np.asarray(w, dtype=np.float32)
    w_a = np.asarray(w_a, dtype=np.float32)
    a = np.asarray(a, dtype=np.float32)

    u = w @ (w_a @ a[2:4])  # [256, 1]
    # params matrix [256, 257 + 8192] -> partition-major image [128, 2, *]
    M = np.concatenate([w, u, node_feats.T], axis=1).astype(ml_dtypes.bfloat16)
    img = M.reshape(2, P, -1).transpose(1, 0, 2)
    pchunks = [
        np.ascontiguousarray(img[:, :, PSPLIT[c] : PSPLIT[c + 1]]) for c in range(3)
    ]

    in_maps = []
    for c in range(NCORES):
        aT_c = Ahat[c * ROWS : (c + 1) * ROWS, :].T  # [8192, 1024]
        aT_img = np.ascontiguousarray(
            aT_c.reshape(NJ, P, ROWS).transpose(1, 0, 2).astype(ml_dtypes.float8_e4m3)
        )
        in_maps.append(
            {"aT": aT_img, "p0": pchunks[0], "p1": pchunks[1], "p2": pchunks[2]}
        )
    return in_maps


def _run(inputs, trace=False, **kwargs):
    if "nc" not in _CACHE:
        _CACHE["nc"] = _build()
    nc = _CACHE["nc"]
    in_maps = _prep_inputs(**inputs)
    res = run_bass_kernel_spmd(
        nc, in_maps, core_ids=list(range(NCORES)), trace=trace, **kwargs
    )
    # out image [128, 8, 256] -> rows (i*128 + p)
    full = np.concatenate(
        [
            res.results[c]["out"].transpose(1, 0, 2).reshape(ROWS, F)
            for c in range(NCORES)
        ],
        axis=0,
    )
    return full, res


def kernel(**inputs) -> np.ndarray:
    out, _ = _run(inputs, trace=False)
    return out


# revision 15
# speedup vs baseline: 1.4686x; 1.4686x over previous
"""GAT/GCN message-passing layer on 8 Trainium2 NeuronCores.

Math: per query node i the reference computes
    e[i,j] = f_src[i] + f_dst[j]   (masked by Ahat>0), attn = softmax_j, then
    out = relu(attn @ h_prime).
f_src[i] is constant along the softmax axis so it cancels; with g = exp(f_dst)
the layer collapses to one GEMM over the adjacency:
    out = relu( (Ahat @ [g*h' | g])[:, :256] / (Ahat @ [g*h' | g])[:, 256:] )
with h' = node_feats @ w and f_dst = node_feats @ (w @ w_a @ a[2:4]).

Sharding: 1D row partition of query nodes; each core owns 1024 output rows and
streams its [8192, 1024] adjacency slice (transposed so the contraction axis j
lands on SBUF partitions).  w/u/node_feats^T are replicated; every core
recomputes the B = [g*h' | g] panel locally.

Key speed tricks vs. the bf16 baseline:
  * The adjacency is binary, so fp8e4 is lossless for it: half the HBM bytes,
    and the whole 8.4MB/core slice fits pinned in SBUF - no streaming ring,
    no refill choreography.  The main GEMM runs with mixed dtypes: fp8 A as
    the stationary operand, bf16 B moving (the PE allows mixed non-fp32
    inputs), so B keeps full bf16 accuracy (end-to-end rel-err ~2.4e-3).
  * All DRAM tensors are laid out as pre-tiled SBUF images (partition-major),
    so every DMA is 128 contiguous descriptors instead of 1024 - descriptor
    generation on the sequencers drops from ~8us to ~1us per load.
  * The prefix (h' panel) is interleaved with the main GEMM: prefix block j
    runs on PSUM banks 6/7 while main j-blocks <= j-3 accumulate in banks
    0-5; i-blocks 6/7 are backfilled after the prefix finishes (all of A is
    resident, so the backfill is pure PE work).

walrus accepts only ONE sync wait per instruction, so the dataflow keeps each
instruction's cross-engine deps on a single engine: all of B-prep lives on
ACT (exp then scale-copy, so the PE wait of the copy is dominated by exp's
and elided), main matmuls wait only on ACT, DMA first-touches are absorbed by
PE nops at chunk boundaries, and the output stores go through gpsimd whose
DMA queues carry no load traffic.
"""

import sys

import ml_dtypes
import numpy as np

sys.path.insert(0, "/opt/trn_rl_repo")

import concourse.bass as bass  # noqa: E402
import concourse.tile as tile  # noqa: E402
from concourse import mybir  # noqa: E402
from concourse.bass_utils import run_bass_kernel_spmd  # noqa: E402
from concourse.tile import add_dep_helper  # noqa: E402

N = 8192
F = 256  # in_features == out_features
FE = F + 1  # h' columns + the g column
NCORES = 8
ROWS = N // NCORES  # 1024 output rows per core
P = 128
NJ = N // P  # 64 contraction blocks
NI = ROWS // P  # 8 output-row blocks per core

BF = mybir.dt.bfloat16
F8 = mybir.dt.float8e4
F32 = mybir.dt.float32

# params image column split: chunk0 = wext + nfT blocks 0..7, then 8..31, 32..63
PSPLIT = [0, FE + 8 * P, FE + 32 * P, FE + NJ * P]
# adjacency chunk split, in j-blocks: small first chunks so the first main
# j-blocks can start while params stream
ASPLIT = [0, 2, 8, 16, 24, 32, 40, 48, 56, 64]

_CACHE = {}


def _build():
    nc = bass.Bass(
        "TRN2",
        target_bir_lowering=False,
        debug=False,
        enable_asserts=True,
        num_devices=NCORES,
    )
    # pre-tiled images (partition-major; see _prep_inputs)
    aT = nc.dram_tensor("aT", [P, NJ, ROWS], F8, kind="ExternalInput").ap()
    p0 = nc.dram_tensor("p0", [P, 2, PSPLIT[1]], BF, kind="ExternalInput").ap()
    p1 = nc.dram_tensor(
        "p1", [P, 2, PSPLIT[2] - PSPLIT[1]], BF, kind="ExternalInput"
    ).ap()
    p2 = nc.dram_tensor(
        "p2", [P, 2, PSPLIT[3] - PSPLIT[2]], BF, kind="ExternalInput"
    ).ap()
    out = nc.dram_tensor("out", [P, NI, F], F32, kind="ExternalOutput").ap()

    with tile.TileContext(nc) as tc:
        _body(tc, aT, (p0, p1, p2), out)
    return nc


def _body(tc, aT, params, out):
    nc = tc.nc
    Exp = mybir.ActivationFunctionType.Exp
    Relu = mybir.ActivationFunctionType.Relu

    with (
        tc.tile_pool(name="consts", bufs=1) as consts,
        tc.tile_pool(name="rpool", bufs=8) as rpool,
        tc.tile_pool(name="psum", bufs=1, space="PSUM") as psum,
    ):
        # ---- SBUF tiles ----------------------------------------------------
        p_sb = [
            consts.tile(
                [P, 2, PSPLIT[c + 1] - PSPLIT[c]], BF, tag=f"p{c}", name=f"p{c}"
            )
            for c in range(3)
        ]
        aT_sb = consts.tile([P, NJ, ROWS], F8, tag="aT")
        Bp = consts.tile([P, NJ, FE], BF, tag="Bp")  # [g*h' | g] panel
        G = consts.tile([P, NJ], F32, tag="G")  # g = exp(f_dst)
        otile = consts.tile([P, NI * F], F32, tag="o")

        def nfT(j, kb):
            """SBUF [128, 128] lhsT view of node_feats^T block j, k-half kb."""
            if j < 8:
                return p_sb[0][:, kb, FE + j * P : FE + (j + 1) * P]
            if j < 32:
                return p_sb[1][:, kb, (j - 8) * P : (j - 7) * P]
            return p_sb[2][:, kb, (j - 32) * P : (j - 31) * P]

        wext = p_sb[0][:, :, 0:FE]

        # ---- loads ----------------------------------------------------------
        # all loads on SYNC (gpsimd's DMA queues stay virgin for the output
        # stores, which otherwise pick up a queue-reuse wait on top of their
        # data wait); each DMA is 128 contiguous descriptors.
        pdma = []
        prev = None
        for c in range(3):
            d = nc.sync.dma_start(p_sb[c][:], params[c][:])
            if prev is not None:
                add_dep_helper(d.ins, prev.ins, sync=False, reason="pdma order")
            prev = d
            pdma.append(d)
        adma = []
        for c in range(len(ASPLIT) - 1):
            lo, hi = ASPLIT[c], ASPLIT[c + 1]
            d = nc.sync.dma_start(aT_sb[:, lo:hi, :], aT[:, lo:hi, :])
            add_dep_helper(d.ins, prev.ins, sync=False, reason="adma order")
            prev = d
            adma.append(d)
        # keep params ahead of the adjacency bulk on the HBM fabric
        add_dep_helper(adma[2].ins, pdma[1].ins, reason="params first")
        add_dep_helper(adma[4].ins, pdma[2].ins, reason="params first")

        # ---- PSUM accumulators ----------------------------------------------
        acc = [
            psum.tile([P, FE], F32, tag=f"acc{i}", name=f"acc{i}") for i in range(NI)
        ]
        # prefix h' borrows banks 6/7 (i-blocks 6/7 are backfilled later)
        hp = [acc[6], acc[7]]

        # first-touch absorbers: a PE nop waits on the DMA so the matmul that
        # follows needs only its ACT wait
        def pe_gate(dma):
            nop = nc.tensor.nop(nofuse=True, hint="dma_gate")
            add_dep_helper(nop.ins, dma.ins, reason="dma gate")
            return nop

        prev_act = None
        last_mm = None

        def prefix(j):
            nonlocal prev_act, last_mm
            h = hp[j % 2]
            for kb in range(2):
                last_mm = nc.tensor.matmul(
                    h[:],
                    lhsT=nfT(j, kb),
                    rhs=wext[:, kb, :],
                    start=(kb == 0),
                    stop=(kb == 1),
                )
            # G[:, j] = exp(f_dst); takes the single PE wait for this bank so
            # the Bp write below needs none
            ex = nc.scalar.activation(G[:, j : j + 1], h[:, F : F + 1], Exp)
            if prev_act is not None:
                add_dep_helper(ex.ins, prev_act.ins, sync=False, reason="act order")
            # Bp[j] = bf16(h' * g)
            bp = nc.scalar.mul(Bp[:, j, 0:F], h[:, 0:F], G[:, j : j + 1])
            add_dep_helper(bp.ins, ex.ins, sync=False, reason="act order")
            prev_act = bp
            if j % 4 == 3:
                # drop this 4-group's g columns into Bp (one strided cast-copy)
                j0 = j - 3
                gh = nc.scalar.copy(Bp[:, j0 : j + 1, F], G[:, j0 : j + 1])
                add_dep_helper(gh.ins, prev_act.ins, sync=False, reason="act order")
                prev_act = gh

        def main_block(j, ilist, start, stop):
            nonlocal last_mm
            for i in ilist:
                last_mm = nc.tensor.matmul(
                    acc[i][:],
                    lhsT=aT_sb[:, j, i * P : (i + 1) * P],
                    rhs=Bp[:, j, :],
                    start=start,
                    stop=stop,
                )

        # ---- interleaved prefix + main stream --------------------------------
        # adjacency chunk c covers j-blocks ASPLIT[c] .. ASPLIT[c+1]-1
        a_gate = {ASPLIT[c]: adma[c] for c in range(len(adma))}
        pe_gate(pdma[0])
        for j in range(NJ):
            if j == 8:
                pe_gate(pdma[1])
            if j == 32:
                pe_gate(pdma[2])
            prefix(j)
            if j >= 3:
                jm = j - 3  # main j-blocks 0..60 during the prefix
                if jm in a_gate:
                    pe_gate(a_gate[jm])
                main_block(jm, range(6), start=(jm == 0), stop=False)
        for jm in range(NJ - 3, NJ):
            main_block(jm, range(6), start=False, stop=(jm == NJ - 1))
        # backfill i-blocks 6/7 (banks 6/7 are free once the prefix drained)
        for j in range(NJ):
            main_block(j, (6, 7), start=(j == 0), stop=(j == NJ - 1))

        # ---- epilogue: out[i] = relu(acc[i][:, :F] / acc[i][:, F]) ----------
        # banks 0..5 finish at the end of the interleaved stream and drain on
        # ACT while the backfill matmuls still run; banks 6/7 drain via DVE.
        stores = []
        banksA = list(range(6))
        denomA = rpool.tile([P, len(banksA)], F32, tag="denomA")
        denom_last = None
        for k, i in enumerate(banksA):
            dc = nc.scalar.copy(denomA[:, k : k + 1], acc[i][:, F : F + 1])
            if denom_last is not None:
                add_dep_helper(dc.ins, denom_last.ins, sync=False, reason="act order")
            denom_last = dc
        recipA = rpool.tile([P, len(banksA)], F32, tag="recipA")
        nc.vector.reciprocal(recipA[:], denomA[:])
        # sacrificial ACT read absorbs the DVE tick for the six fused relus
        sacA = rpool.tile([P, len(banksA)], F32, tag="sacA")
        sa = nc.scalar.copy(sacA[:], recipA[:])
        add_dep_helper(sa.ins, denom_last.ins, sync=False, reason="act order")
        last_relu = sa
        for k, i in enumerate(banksA):
            o = otile[:, i * F : (i + 1) * F]
            rl = nc.scalar.activation(
                o, acc[i][:, 0:F], Relu, scale=recipA[:, k : k + 1]
            )
            add_dep_helper(rl.ins, last_relu.ins, sync=False, reason="act order")
            last_relu = rl
        # gpsimd nop absorbs the ACT dep so the store itself carries only its
        # DMA-queue wait
        gnop = nc.gpsimd.nop(nofuse=True, hint="storeA_gate")
        add_dep_helper(gnop.ins, last_relu.ins, reason="storeA gate")
        stores.append(nc.gpsimd.dma_start(out[:, 0:6, :], otile[:, 0 : 6 * F]))
        add_dep_helper(stores[-1].ins, gnop.ins, sync=False, reason="after gate")

        # banks 6/7 (backfill): DVE path
        denomB = rpool.tile([P, 2], F32, tag="denomB")
        for k, i in enumerate([6, 7]):
            dc = nc.scalar.copy(denomB[:, k : k + 1], acc[i][:, F : F + 1])
            add_dep_helper(
                dc.ins, (denom_last or dc).ins, sync=False, reason="act order"
            )
            denom_last = dc
        recipB = rpool.tile([P, 2], F32, tag="recipB")
        nc.vector.reciprocal(recipB[:], denomB[:])
        rscrB = rpool.tile([P, 2], F32, tag="rscrB")
        nc.vector.tensor_copy(rscrB[:], recipB[:])
        last_dve = None
        for k, i in enumerate([6, 7]):
            o = otile[:, i * F : (i + 1) * F]
            nc.vector.tensor_scalar_mul(o, acc[i][:, 0:F], recipB[:, k : k + 1])
            last_dve = nc.vector.tensor_scalar_max(o, o, 0.0)
        gnop = nc.gpsimd.nop(nofuse=True, hint="storeB_gate")
        add_dep_helper(gnop.ins, last_dve.ins, reason="storeB gate")
        stores.append(nc.gpsimd.dma_start(out[:, 6:8, :], otile[:, 6 * F :]))
        add_dep_helper(stores[-1].ins, gnop.ins, sync=False, reason="after gate")

        # funnel every proc's final tick into SP via single-wait nops so the
        # kernel-tail drain has nothing left to wait on (every DMA queue's
        # final count included, else the drain aggregates 10+ waits)
        for dep in [*pdma, *adma, *stores, last_mm, last_relu, last_dve, prev_act]:
            nop = nc.sync.nop(nofuse=True, hint="tail_funnel")
            add_dep_helper(nop.ins, dep.ins, reason="tail funnel")


def _prep_inputs(node_feats, Ahat, w, w_a, a):
    node_feats = np.asarray(node_feats, dtype=np.float32)
    Ahat = np.asarray(Ahat, dtype=np.float32)
    w = np.asarray(w, dtype=np.float32)
    w_a = np.asarray(w_a, dtype=np.float32)
    a = np.asarray(a, dtype=np.float32)

    u = w @ (w_a @ a[2:4])  # [256, 1]
    # params matrix [256, 257 + 8192] -> partition-major image [128, 2, *]
    M = np.concatenate([w, u, node_feats.T], axis=1).astype(ml_dtypes.bfloat16)
    img = M.reshape(2, P, -1).transpose(1, 0, 2)
    pchunks = [
        np.ascontiguousarray(img[:, :, PSPLIT[c] : PSPLIT[c + 1]]) for c in range(3)
    ]

    in_maps = []
    for c in range(NCORES):
        aT_c = Ahat[c * ROWS : (c + 1) * ROWS, :].T  # [8192, 1024]
        aT_img = np.ascontiguousarray(
            aT_c.reshape(NJ, P, ROWS).transpose(1, 0, 2).astype(ml_dtypes.float8_e4m3)
        )
        in_maps.append(
            {"aT": aT_img, "p0": pchunks[0], "p1": pchunks[1], "p2": pchunks[2]}
        )
    return in_maps


def _run(inputs, trace=False, **kwargs):
    if "nc" not in _CACHE:
        _CACHE["nc"] = _build()
    nc = _CACHE["nc"]
    in_maps = _prep_inputs(**inputs)
    res = run_bass_kernel_spmd(
        nc, in_maps, core_ids=list(range(NCORES)), trace=trace, **kwargs
    )
    # out image [128, 8, 256] -> rows (i*128 + p)
    full = np.concatenate(
        [
            res.results[c]["out"].transpose(1, 0, 2).reshape(ROWS, F)
            for c in range(NCORES)
        ],
        axis=0,
    )
    return full, res


def kernel(**inputs) -> np.ndarray:
    out, _ = _run(inputs, trace=False)
    return out
